# revision 11
# baseline (speedup 1.0000x reference)
"""DAGNN-conv (3-hop mean-aggregation GNN + gated hop combine) on 8 trn2 cores.

Environment law (measured): ~40us per UNIQUE engine instruction; re-execution
via For_i hardware loops is ~free; DMA/collective (sequencer) instructions are
cheap.  So the kernel is built from a minimal set of instructions with rolled
loops and mega-APs:

  - Nodes row-sharded across 8 cores (1250 each, padded 1264/core so the
    AllGather blocks tile 10112 = 79*128 rows).
  - Per-hop h' = D^-1 A h as dense matmul; per-core A^T (dst-sharded,
    [10112 x 1280]) stored as fp8e4m3 counts (exact), RESIDENT in SBUF.
  - h carried as bf16 hi/lo split (h = hi+lo) -> PE products exact, PSUM
    accumulates fp32 => near-fp32 accuracy.
  - k-loop (80 K-tiles, 2/iter) is a single rolled For_i per hop: 20 matmul
    instructions + 1 copy-through of A strips to a fixed staging buffer
    (lhsT cannot take register offsets; ACT copies strips bitcast-as-f32).
  - PSUM accumulation groups are opened by K=1 zeroing matmuls (start=True)
    so all in-loop matmuls run start=False.
  - hi|lo own-shard block AllGathered between hops (straight-line;
    collectives inside For_i do not execute on this runtime).
  - Gate scores/softmax/combine: a handful of mega-AP DVE/ACT ops.

kernel(**inputs) takes FULL inputs (reference.setup_inputs() keys) and
returns the FULL [10000, 128] float32 output.
"""
import numpy as np
import sys

sys.path.insert(0, "/opt/trn_rl_repo")

import ml_dtypes  # noqa: E402

from concourse import bass, bacc, tile, mybir  # noqa: E402
from concourse.bass_utils import run_bass_kernel_spmd  # noqa: E402

N = 10000
C = 128
CORES = 8
OWN = 1250          # real nodes per core
BLK = 1264          # allgather block rows per core (8*1264 = 10112)
NP = CORES * BLK    # 10112 padded global rows
KT = NP // 128      # 79 K-tiles
KTP = 80            # padded K-tiles (strip 79 = zeros)
KTA = 82            # A strips incl. junk prefetch area
MT = 10             # M-tiles per core (1280 rows)
OWNP = MT * 128
STEPS = 3

BF16 = ml_dtypes.bfloat16
FP8 = ml_dtypes.float8_e4m3

_NC_CACHE = {}


def _g_rows(n):
    return BLK * (n // OWN) + (n % OWN)


def _build_nc():
    f32 = mybir.dt.float32
    bf16 = mybir.dt.bfloat16
    fp8 = mybir.dt.float8e4
    add = mybir.AluOpType.add
    sub = mybir.AluOpType.subtract
    mult = mybir.AluOpType.mult
    AF = mybir.ActivationFunctionType

    nc = bacc.Bacc("TRN2", target_bir_lowering=False, debug=False,
                   num_devices=CORES)

    # a_in[p, k, q] = count[dst own q, src_pad k*128+p]; strips >= 79 zero.
    a_in = nc.dram_tensor("a_in", [128, KTA, OWNP], fp8,
                          kind="ExternalInput").ap()
    x_cat = nc.dram_tensor("x_cat", [128, KTP, 256], bf16,
                           kind="ExternalInput").ap()
    x_own = nc.dram_tensor("x_own", [128, MT, 128], f32,
                           kind="ExternalInput").ap()
    invdb_in = nc.dram_tensor("invdb", [128, MT, 128], f32,
                              kind="ExternalInput").ap()
    wb_in = nc.dram_tensor("wb", [128, MT, 128], f32,
                           kind="ExternalInput").ap()
    out = nc.dram_tensor("out", [OWN, C], f32, kind="ExternalOutput").ap()

    with tile.TileContext(nc) as tc:
        with (
            tc.tile_pool(name="big", bufs=1) as big,
            tc.tile_pool(name="work", bufs=1) as work,
            tc.tile_pool(name="psum", bufs=1, space="PSUM") as psum,
            tc.tile_pool(name="dram", bufs=1, space="DRAM") as dram,
        ):
            a_res = big.tile([128, KTA, OWNP], fp8)          # ~105KB/part
            nc.sync.dma_start(out=a_res[:], in_=a_in[:])
            rhs_tab = big.tile([128, KTP, 256], bf16)        # 40KB/part
            nc.sync.dma_start(out=rhs_tab[:], in_=x_cat[:])

            invdb = work.tile([128, MT, 128], f32)
            nc.sync.dma_start(out=invdb[:], in_=invdb_in[:])
            wb = work.tile([128, 1, MT, 128], f32)
            nc.sync.dma_start(out=wb[:, 0], in_=wb_in[:])
            h_own = work.tile([128, 4, MT, 128], f32)        # 20KB/part
            nc.sync.dma_start(out=h_own[:, 0], in_=x_own[:])

            zcol = work.tile([1, 128], f32)
            nc.vector.memset(zcol[:], 0.0)
            zrow = work.tile([1, 512], f32)
            nc.vector.memset(zrow[:], 0.0)

            # staging buffer for 2 A strips (lhsT needs static offsets)
            abuf = work.tile([128, 2, OWNP], fp8)
            nc.scalar.activation(abuf[:].bitcast(f32),
                                 a_res[:, 0:2, :].bitcast(f32), AF.Copy)

            cc_src = work.tile([128, MT, 256], bf16, tag="xchg")
            lo_tmp = work.tile([128, MT, 128], f32, tag="ptmp")
            pt = psum.tile([128, MT, 256], f32)              # 10KB/part, 5 banks

            cc_in = dram.tile([BLK, 256], bf16, tag="cc_in")
            cc_out = dram.tile([NP, 256], bf16, tag="cc_out")

            for s in range(1, STEPS + 1):
                # open fp32 accumulation: zero PSUM + clear has_written
                pt_flat = pt[:].rearrange("p m c -> p (m c)")
                for z in range(5):
                    nc.tensor.matmul(
                        pt_flat[:, z * 512:(z + 1) * 512],
                        lhsT=zcol[:], rhs=zrow[:], start=True, stop=True)
                with tc.For_i(0, KTP, 2) as k:
                    for j in range(2):
                        for m in range(MT):
                            nc.tensor.matmul(
                                pt[:, m, :],
                                lhsT=abuf[:, j, m * 128:(m + 1) * 128],
                                rhs=rhs_tab[:, bass.ds(k + j, 1), :],
                                start=False, stop=True)
                    # prefetch strips k+2, k+3 for the next iteration
                    nc.scalar.activation(
                        abuf[:].bitcast(f32),
                        a_res[:, bass.ds(k + 2, 2), :].bitcast(f32), AF.Copy)

                # h_s = (hi_sum + lo_sum) * inv_deg
                nc.scalar.activation(lo_tmp[:],
                                     pt[:].rearrange("p m (h c) -> p m h c", h=2)
                                     [:, :, 1, :], AF.Copy)
                nc.vector.tensor_tensor(
                    lo_tmp[:],
                    pt[:].rearrange("p m (h c) -> p m h c", h=2)[:, :, 0, :],
                    lo_tmp[:], op=add)
                nc.vector.tensor_tensor(h_own[:, s], lo_tmp[:], invdb[:], op=mult)

                if s < STEPS:
                    # bf16 hi/lo split of own shard, exchange, reload rhs_tab
                    nc.scalar.activation(
                        cc_src[:].rearrange("p m (h c) -> p m h c", h=2)
                        [:, :, 0, :], h_own[:, s], AF.Copy)
                    nc.vector.tensor_tensor(
                        cc_src[:].rearrange("p m (h c) -> p m h c", h=2)
                        [:, :, 1, :], h_own[:, s],
                        cc_src[:].rearrange("p m (h c) -> p m h c", h=2)
                        [:, :, 0, :], op=sub)
                    nc.sync.dma_start(
                        out=cc_in[0:1152, :].rearrange("(m p) j -> p m j", p=128),
                        in_=cc_src[:, 0:9, :])
                    nc.sync.dma_start(out=cc_in[1152:BLK, :],
                                      in_=cc_src[0:112, 9, :])
                    nc.gpsimd.collective_compute(
                        "AllGather", mybir.AluOpType.bypass,
                        replica_groups=[list(range(CORES))],
                        ins=[cc_in.opt()], outs=[cc_out.opt()])
                    nc.sync.dma_start(
                        out=rhs_tab[:, 0:KT, :],
                        in_=cc_out[:].rearrange("(k p) j -> p k j", p=128))
                    # re-seed the staging buffer with strips 0,1
                    nc.scalar.activation(abuf[:].bitcast(f32),
                                         a_res[:, 0:2, :].bitcast(f32), AF.Copy)

            # ---- gate scores, softmax over 4 hop outputs, combine ----
            prod = work.tile([128, 4, MT, 128], f32, tag="ptmp")
            sc = work.tile([128, 4, MT], f32)
            e = work.tile([128, 4, MT], f32)
            z = work.tile([128, MT], f32)
            r = work.tile([128, 1, MT], f32)
            w4 = work.tile([128, 4, MT, 1], f32)
            acc = work.tile([128, MT, 128], f32, tag="xchg")

            nc.vector.tensor_tensor(prod[:], h_own[:],
                                    wb[:].broadcast_to([128, 4, MT, 128]),
                                    op=mult)
            nc.vector.tensor_reduce(sc[:], prod[:],
                                    axis=mybir.AxisListType.X, op=add)
            nc.scalar.activation(e[:], sc[:], AF.Exp)
            nc.vector.tensor_reduce(z[:], e[:].rearrange("p t m -> p m t"),
                                    axis=mybir.AxisListType.X, op=add)
            nc.vector.reciprocal(r[:, 0], z[:])
            nc.vector.tensor_tensor(w4[:, :, :, 0], e[:],
                                    r[:].broadcast_to([128, 4, MT]), op=mult)
            nc.vector.tensor_tensor(prod[:], h_own[:],
                                    w4[:].broadcast_to([128, 4, MT, 128]),
                                    op=mult)
            nc.vector.tensor_reduce(
                acc[:], prod[:].rearrange("p t m c -> p m c t"),
                axis=mybir.AxisListType.X, op=add)

            nc.sync.dma_start(
                out=out[0:1152, :].rearrange("(m p) j -> p m j", p=128),
                in_=acc[:, 0:9, :])
            nc.sync.dma_start(out=out[1152:OWN, :], in_=acc[0:98, 9, :])

    nc.compile()
    return nc


def _prep_inputs(x, edge_index, gate_w):
    x = np.asarray(x, dtype=np.float32)
    ei = np.asarray(edge_index)
    src = ei[0].astype(np.int64)
    dst = ei[1].astype(np.int64)
    w = np.asarray(gate_w, dtype=np.float32).reshape(C)

    deg = np.bincount(dst, minlength=N).astype(np.float32)
    inv_deg = np.where(deg > 0, 1.0 / np.maximum(deg, 1), 0.0).astype(np.float32)

    x_pad = np.zeros((NP, C), dtype=np.float32)
    x_pad[_g_rows(np.arange(N))] = x
    hi = x_pad.astype(BF16)
    lo = (x_pad - hi.astype(np.float32)).astype(BF16)
    cat = np.concatenate([hi, lo], axis=1)                  # [NP, 256] bf16
    x_cat = np.zeros((128, KTP, 256), dtype=BF16)
    x_cat[:, :KT, :] = cat.reshape(KT, 128, 256).transpose(1, 0, 2)

    src_pad = _g_rows(src)
    wb = np.ascontiguousarray(
        np.broadcast_to(w, (128, MT, C))).astype(np.float32)

    in_maps = []
    for c in range(CORES):
        lo_n, hi_n = OWN * c, OWN * (c + 1)
        sel = (dst >= lo_n) & (dst < hi_n)
        d_own = (dst[sel] - lo_n).astype(np.int64)
        s_pad = src_pad[sel]
        counts = np.bincount(d_own * NP + s_pad,
                             minlength=OWNP * NP).reshape(OWNP, NP)
        assert counts.max() <= 16, "edge multiplicity too large for fp8"
        a_host = np.zeros((128, KTA, OWNP), dtype=FP8)
        a_host[:, :KT, :] = counts.reshape(OWNP, KT, 128).transpose(2, 1, 0)

        xo = np.zeros((OWNP, C), dtype=np.float32)
        xo[:OWN] = x[lo_n:hi_n]
        x_own = np.ascontiguousarray(xo.reshape(MT, 128, C).transpose(1, 0, 2))

        dv = np.zeros(OWNP, dtype=np.float32)
        dv[:OWN] = inv_deg[lo_n:hi_n]
        invdb = np.ascontiguousarray(
            np.broadcast_to(dv.reshape(MT, 128).T[:, :, None],
                            (128, MT, C))).astype(np.float32)

        in_maps.append({
            "a_in": a_host,
            "x_cat": x_cat,
            "x_own": x_own,
            "invdb": invdb,
            "wb": wb,
        })
    return in_maps


LAST_EXEC_NS = None


def kernel(x, edge_index, gate_w, gate_b):
    # gate_b shifts every hop's score equally -> softmax-invariant; unused.
    global LAST_EXEC_NS
    import time as _time

    if "nc" not in _NC_CACHE:
        _NC_CACHE["nc"] = _build_nc()
    nc = _NC_CACHE["nc"]

    in_maps = _prep_inputs(x, edge_index, gate_w)
    t0 = _time.time()
    res = run_bass_kernel_spmd(nc, in_maps, list(range(CORES)))
    # NTFF profiling is unavailable under this axon client; this wall time
    # includes host<->device transfer of ~110MB of inputs on top of the
    # ~120ms NEFF execution (measured against a null kernel).
    LAST_EXEC_NS = int((_time.time() - t0) * 1e9)
    out = np.concatenate([res.results[c]["out"] for c in range(CORES)], axis=0)
    return out.astype(np.float32)
